# revision 1
# baseline (speedup 1.0000x reference)
"""Multi-head attention (B=4, T=2048, D=1024, H=16) on 8 TRN2 NeuronCores.

Sharding: core c = (batch b = c//2, head-group g = c%2). Each core computes
QKV projections for its 8 heads, attention, then (after a pairwise AllGather
of the per-head attention outputs, 2MB bf16) the full output projection for
its batch. Host assembles full[b] from core 2b.

Kernel layout highlights:
  - All matmuls bf16, K=64 row-tiled (64x128 tile mode throughout; no mode
    switches): projections, scores, attn@V, out-proj.
  - Scores are computed transposed (S^T = K_mat @ Q^T per 128-row T_k chunk)
    so exp() output lands directly in the [T_k, T_q] layout attn@V needs.
  - exp on ScalarE reads scores straight out of PSUM [128,1024] (2 banks)
    and writes bf16 to SBUF.
  - V carries 64 extra all-ones columns: attn@V PSUM rows 64..127 are the
    softmax denominator replicated across 64 partitions, so the normalize
    (reciprocal + multiply) is partition-aligned on VectorE - no broadcast.
  - 1/sqrt(d_k) and biases are folded host-side / into PSUM evacuation.
"""

import numpy as np
import ml_dtypes

import concourse.bass as bass
import concourse.tile as tile
from concourse import mybir
from concourse.bass_utils import run_bass_kernel_spmd

BF16 = mybir.dt.bfloat16
F32 = mybir.dt.float32
NPBF16 = ml_dtypes.bfloat16

N_CORES = 8
B, T, D, H = 4, 2048, 1024, 16
DK = D // H          # 64
HL = H // 2          # heads per core (8)
NHP = HL // 2        # head pairs per core (4)
NJP = D // 128       # input-dim 128-chunks (8)
NOC = (D // 2) // 128  # per-core qkv out-dim 128-chunks (4)
NTT = T // 512       # T 512-tiles (4)
NTC = T // 128       # T 128-chunks (16)

_uid = [0]


def _split_sync_commands(nc, max_waits=1, max_updates=1):
    """This walrus build allows only one sync wait/update command on
    sequencer-only (TPB_CTRL) instructions like Drain/NoOp; Tile's kernel
    tail drain carries one wait per logical processor. Split the excess onto
    adjacent same-engine NoOps (program order makes this equivalent)."""
    for func in nc.m.functions:
        for block in func.blocks:
            out = []
            changed = False
            for inst in block.instructions:
                si = inst.sync_info
                if si is None:
                    out.append(inst)
                    continue
                is_dma = "DMA" in type(inst).__name__.upper() or "DMA" in str(
                    getattr(inst, "opcode", "")).upper()
                waits = list(si.on_wait or [])
                # DMA completion increments must stay on the DMA instruction;
                # waits can always move to a preceding same-engine NoOp.
                updates = list(si.on_update or [])
                if is_dma:
                    n_up = len(updates)
                    updates_keep, updates = updates, []
                else:
                    updates_keep = None
                pre, post = [], []
                while len(waits) > max_waits:
                    chunk, waits = waits[:max_waits], waits[max_waits:]
                    _uid[0] += 1
                    pre.append(mybir.InstNoOp(
                        name=f"I-syncsplit-{_uid[0]}", engine=inst.engine,
                        bass_nofuse=True,
                        sync_info=mybir.SyncInfo(on_wait=chunk, on_update=[])))
                while len(updates) > max_updates:
                    chunk, updates = updates[:max_updates], updates[max_updates:]
                    _uid[0] += 1
                    post.append(mybir.InstNoOp(
                        name=f"I-syncsplit-{_uid[0]}", engine=inst.engine,
                        bass_nofuse=True,
                        sync_info=mybir.SyncInfo(on_wait=[], on_update=chunk)))
                if updates_keep is not None:
                    updates = updates_keep
                if pre or post:
                    inst.sync_info = mybir.SyncInfo(on_wait=waits, on_update=updates)
                    changed = True
                out.extend(pre)
                out.append(inst)
                out.extend(post)
            if changed:
                block.instructions = out


def build_nc(split_sync=True):
    nc = bass.Bass("TRN2", target_bir_lowering=False, debug=False,
                   num_devices=N_CORES)

    xt_ext = nc.dram_tensor("xt", [NJP, 128, T], BF16, kind="ExternalInput").ap()
    wq_ext = nc.dram_tensor("wq", [128, NJP, NOC, 128], BF16, kind="ExternalInput").ap()
    wk_ext = nc.dram_tensor("wk", [128, NJP, NOC, 128], BF16, kind="ExternalInput").ap()
    wv_ext = nc.dram_tensor("wv", [128, NJP, 512], BF16, kind="ExternalInput").ap()
    wo_ext = nc.dram_tensor("wo", [128, 2, NHP, 2, 512], BF16, kind="ExternalInput").ap()
    bq_ext = nc.dram_tensor("bq", [128, NOC], F32, kind="ExternalInput").ap()
    bk_ext = nc.dram_tensor("bk", [128, NOC], F32, kind="ExternalInput").ap()
    bv_ext = nc.dram_tensor("bv", [1, 512], F32, kind="ExternalInput").ap()
    bo_ext = nc.dram_tensor("bo", [1, D], F32, kind="ExternalInput").ap()
    out_ext = nc.dram_tensor("out", [T, D], F32, kind="ExternalOutput").ap()

    with tile.TileContext(nc) as tc:
        with (
            tc.tile_pool(name="persist", bufs=1) as persist,
            tc.tile_pool(name="epool", bufs=6) as epool,
            tc.tile_pool(name="evac", bufs=2) as evac,
            tc.tile_pool(name="outstage", bufs=2) as outstage,
            tc.tile_pool(name="ps", bufs=2, space="PSUM") as ps_pool,
            tc.tile_pool(name="po", bufs=4, space="PSUM") as po_pool,
            tc.tile_pool(name="dram", bufs=1, space="DRAM") as dram,
        ):
            # ---- weights / biases in ----
            wq_sb = persist.tile([128, NJP, NOC, 128], BF16, tag="wq", name="wq")
            wk_sb = persist.tile([128, NJP, NOC, 128], BF16, tag="wk", name="wk")
            wv_sb = persist.tile([128, NJP, 512], BF16, tag="wv", name="wv")
            wo_sb = persist.tile([128, 2, NHP, 2, 512], BF16, tag="wo", name="wo")
            bq_sb = persist.tile([128, NOC], F32, tag="bq", name="bq")
            bk_sb = persist.tile([128, NOC], F32, tag="bk", name="bk")
            bv_sb = persist.tile([128, 512], F32, tag="bv", name="bv")
            bo_sb = persist.tile([128, D], F32, tag="bo", name="bo")
            nc.sync.dma_start(out=wv_sb[:], in_=wv_ext[:])
            # broadcast along partitions (stride-0 partition dim on the DRAM side)
            nc.sync.dma_start(
                out=bv_sb[:],
                in_=bass.AP(tensor=bv_ext.tensor, offset=bv_ext.offset,
                            ap=[[0, 128]] + list(bv_ext.ap[1:])))
            nc.sync.dma_start(
                out=bo_sb[:],
                in_=bass.AP(tensor=bo_ext.tensor, offset=bo_ext.offset,
                            ap=[[0, 128]] + list(bo_ext.ap[1:])))

            xt_sb = []
            for p in range(NJP):
                t_ = persist.tile([128, T], BF16, tag=f"xt{p}", name=f"xt{p}")
                nc.sync.dma_start(out=t_[:], in_=xt_ext[p])
                xt_sb.append(t_)
            nc.sync.dma_start(out=wk_sb[:], in_=wk_ext[:])
            nc.sync.dma_start(out=wq_sb[:], in_=wq_ext[:])
            nc.sync.dma_start(out=bq_sb[:], in_=bq_ext[:])
            nc.sync.dma_start(out=bk_sb[:], in_=bk_ext[:])
            nc.sync.dma_start(out=wo_sb[:], in_=wo_ext[:])

            qt_sb = [persist.tile([128, T], BF16, tag=f"qt{i}", name=f"qt{i}") for i in range(NOC)]
            kt_sb = [persist.tile([128, T], BF16, tag=f"kt{i}", name=f"kt{i}") for i in range(NOC)]
            vh_sb = [persist.tile([128, HL, 128], BF16, tag=f"vh{i}", name=f"vh{i}") for i in range(NTC)]
            ot_a = [persist.tile([128, NHP, 512], BF16, tag=f"ot_a{q}", name=f"ot_a{q}")
                    for q in range(NTT)]
            ot_b = [persist.tile([128, NHP, 512], BF16, tag=f"ot_b{q}", name=f"ot_b{q}")
                    for q in range(NTT)]

            # ---- projections: v first, then q/k per head-pair (oc-major)
            # so attention for hp=0 can start while later hps still project.
            for tcc in range(NTC):
                psv = po_pool.tile([128, 512], F32, tag="po", name="po")
                for j in range(NJP):
                    nc.tensor.matmul(
                        psv[:],
                        lhsT=xt_sb[j][:, tcc * 128:(tcc + 1) * 128],
                        rhs=wv_sb[:, j, :],
                        start=(j == 0), stop=(j == NJP - 1))
                nc.vector.tensor_tensor(
                    vh_sb[tcc][:, :, 0:64],
                    psv[:].rearrange("p (h d) -> p h d", h=HL),
                    bv_sb[:].rearrange("p (h d) -> p h d", h=HL),
                    mybir.AluOpType.add)
                nc.vector.memset(vh_sb[tcc][:, :, 64:128], 1.0)
            for oc in range(NOC):
                for w_sb, b_sb, dst in ((wk_sb, bk_sb, kt_sb), (wq_sb, bq_sb, qt_sb)):
                    for tt in range(NTT):
                        psq = po_pool.tile([128, 512], F32, tag="po", name="po")
                        for j in range(NJP):
                            nc.tensor.matmul(
                                psq[:],
                                lhsT=w_sb[:, j, oc, :],
                                rhs=xt_sb[j][:, tt * 512:(tt + 1) * 512],
                                start=(j == 0), stop=(j == NJP - 1))
                        nc.vector.tensor_scalar_add(
                            dst[oc][:, tt * 512:(tt + 1) * 512], psq[:],
                            b_sb[:, oc:oc + 1])

            # ---- attention (per T_q 512-tile, per head-pair) ----
            for tq in range(NTT):
                for hp in range(NHP):
                    po4 = [po_pool.tile([128, 512], F32, tag="po", name="po") for _ in range(4)]
                    # software-pipelined: scores/exp(kc) issue before attnV(kc-1)
                    # so the PE queue never head-of-line blocks ScalarE's feed.
                    e_tiles = [None] * NTC

                    def emit_scores(kc):
                        ps = ps_pool.tile([128, 1024], F32, tag="ps", name="ps")
                        for h2 in (0, 1):
                            nc.tensor.matmul(
                                ps[:, h2 * 512:(h2 + 1) * 512],
                                lhsT=kt_sb[hp][h2 * 64:(h2 + 1) * 64, kc * 128:(kc + 1) * 128],
                                rhs=qt_sb[hp][h2 * 64:(h2 + 1) * 64, tq * 512:(tq + 1) * 512],
                                start=True, stop=True,
                                tile_position=(h2 * 64, 0))
                        e_t = epool.tile([128, 1024], BF16, tag="e", name="e")
                        nc.scalar.activation(e_t[:], ps[:],
                                             mybir.ActivationFunctionType.Exp)
                        e_tiles[kc] = e_t

                    def emit_attnv(kc):
                        e_t = e_tiles[kc]
                        for h2 in (0, 1):
                            for half in (0, 1):
                                nc.tensor.matmul(
                                    po4[2 * h2 + half][:],
                                    lhsT=vh_sb[kc][half * 64:(half + 1) * 64, 2 * hp + h2, :],
                                    rhs=e_t[half * 64:(half + 1) * 64, h2 * 512:(h2 + 1) * 512],
                                    start=(kc == 0), stop=(kc == NTC - 1),
                                    tile_position=(half * 64, 0))

                    emit_scores(0)
                    for kc in range(1, NTC):
                        emit_scores(kc)
                        emit_attnv(kc - 1)
                    emit_attnv(NTC - 1)
                    for h2 in (0, 1):
                        o0, o1 = po4[2 * h2], po4[2 * h2 + 1]
                        c1 = evac.tile([128, 512], F32, tag="c1", name="c1")
                        nc.vector.tensor_copy(c1[:], o1[:])
                        sm = evac.tile([128, 512], F32, tag="sm", name="sm")
                        nc.vector.tensor_add(sm[:], o0[:], c1[:])
                        rr = evac.tile([64, 512], F32, tag="rr", name="rr")
                        nc.vector.reciprocal(rr[:], sm[64:128, :])
                        nc.vector.tensor_mul(
                            ot_a[tq][h2 * 64:(h2 + 1) * 64, hp, :],
                            sm[0:64, :], rr[:])
                # exchange this T_q quarter (all head-pairs) while later
                # quarters are still computing
                oT_in = dram.tile([128, NHP, 512], BF16, name=f"oT_in{tq}")
                oT_out = dram.tile([2, 128, NHP, 512], BF16, name=f"oT_out{tq}")
                nc.sync.dma_start(out=oT_in[:], in_=ot_a[tq][:])
                nc.gpsimd.collective_compute(
                    "AllGather",
                    mybir.AluOpType.bypass,
                    ins=[oT_in.opt()],
                    outs=[oT_out.opt()],
                    replica_groups=[[0, 1], [2, 3], [4, 5], [6, 7]],
                )
                nc.sync.dma_start(out=ot_a[tq][:], in_=oT_out[0])
                nc.sync.dma_start(out=ot_b[tq][:], in_=oT_out[1])

            # ---- output projection: full batch rows, both head groups ----
            for tcc in range(NTC):
                for nb in range(2):
                    pso = po_pool.tile([128, 512], F32, tag="po", name="po")
                    first = True
                    q, tl = tcc // 4, tcc % 4
                    for src_i, ot_sb in ((0, ot_a), (1, ot_b)):
                        for hp in range(NHP):
                            nc.tensor.matmul(
                                pso[:],
                                lhsT=ot_sb[q][:, hp, tl * 128:(tl + 1) * 128],
                                rhs=wo_sb[:, src_i, hp, nb, :],
                                start=first,
                                stop=(src_i == 1 and hp == NHP - 1))
                            first = False
                    ost = outstage.tile([128, 512], F32, tag="ost", name="ost")
                    nc.vector.tensor_add(ost[:], pso[:], bo_sb[:, nb * 512:(nb + 1) * 512])
                    nc.sync.dma_start(
                        out=out_ext[tcc * 128:(tcc + 1) * 128, nb * 512:(nb + 1) * 512],
                        in_=ost[:])

    if split_sync:
        _split_sync_commands(nc)
    return nc


_NC_CACHE = {}


def _get_nc():
    if "nc" not in _NC_CACHE:
        _NC_CACHE["nc"] = build_nc()
    return _NC_CACHE["nc"]


def _prep_core_inputs(x, Wq, bq, Wk, bk, Wv, bv, Wo, bo):
    """Host-side sharding + layout. Returns in_maps list (8 cores)."""
    x = np.asarray(x, np.float32)
    s = 1.0 / np.sqrt(np.float32(DK))
    Wq_s, bq_s = np.asarray(Wq, np.float32) * s, np.asarray(bq, np.float32) * s
    Wk_f, bk_f = np.asarray(Wk, np.float32), np.asarray(bk, np.float32)
    Wv_f, bv_f = np.asarray(Wv, np.float32), np.asarray(bv, np.float32)
    Wo_f, bo_f = np.asarray(Wo, np.float32), np.asarray(bo, np.float32)

    # Wo rows regrouped to the on-device O^T layout:
    # [src group, hp, h2, 64] rows -> partitions h2*64+r, free [src, hp, nb, col]
    wo_dev = (Wo_f.reshape(2, NHP, 2, 64, 2, 512)
              .transpose(2, 3, 0, 1, 4, 5)          # [h2, r, src, hp, nb, col]
              .reshape(128, 2, NHP, 2, 512)).astype(NPBF16)
    bo_dev = bo_f.reshape(1, D)

    in_maps = []
    for c in range(N_CORES):
        b, g = c // 2, c % 2
        cols = slice(g * 512, (g + 1) * 512)
        wq_g, bq_g = Wq_s[:, cols], bq_s[cols]
        wk_g, bk_g = Wk_f[:, cols], bk_f[cols]
        wv_g, bv_g = Wv_f[:, cols], bv_f[cols]

        xt_dev = np.ascontiguousarray(x[b].T).reshape(NJP, 128, T).astype(NPBF16)

        def wqk_dev(w):
            # [jp, r, oc, c] -> partitions r, free [jp, oc, c]
            return np.ascontiguousarray(
                w.reshape(NJP, 128, NOC, 128).transpose(1, 0, 2, 3)).astype(NPBF16)

        wv_dev = np.ascontiguousarray(
            wv_g.reshape(NJP, 128, 512).transpose(1, 0, 2)).astype(NPBF16)

        in_maps.append({
            "xt": xt_dev,
            "wq": wqk_dev(wq_g), "wk": wqk_dev(wk_g), "wv": wv_dev,
            "wo": wo_dev,
            "bq": np.ascontiguousarray(bq_g.reshape(NOC, 128).T),
            "bk": np.ascontiguousarray(bk_g.reshape(NOC, 128).T),
            "bv": bv_g.reshape(1, 512),
            "bo": bo_dev,
        })
    return in_maps


def kernel(x, Wq, bq, Wk, bk, Wv, bv, Wo, bo, _trace=False):
    nc = _get_nc()
    in_maps = _prep_core_inputs(x, Wq, bq, Wk, bk, Wv, bv, Wo, bo)
    res = run_bass_kernel_spmd(nc, in_maps, core_ids=list(range(N_CORES)),
                               trace=_trace)
    out = np.empty((B, T, D), np.float32)
    for b in range(B):
        out[b] = res.results[2 * b]["out"]
    if _trace:
        kernel.last_result = res
    return out

